# revision 13
# baseline (speedup 1.0000x reference)
"""Block-diagonal rotation (COB) kernel for Trainium2, 8 NeuronCores.

Computes out[..., block_i] = x[..., block_i] @ W_i.T for 8 square blocks of
sizes [512, 1024, 256, 768, 384, 640, 128, 384] (features sum to 4096),
x shape (4, 2048, 4096) fp32.

Strategy:
  - Pure data-parallel over rows: 8192 rows are split 8 ways (1024 rows/core).
    Each core gets all (host-pre-transposed) weights.
  - Weights are DMA'd once into SBUF and stay resident as float32r
    (TRN2's fast 4-byte matmul dtype: 1 cycle/row vs 4 for fp32,
    ~1.5e-4 max rel err at these contraction depths).
  - x tiles [128, 4096] are DMA'd naturally (rows on partitions), transposed
    128x128 on the TensorEngine (transpose mode), PSUM->SBUF copied by the
    VectorEngine, then used as the stationary operand of f32r matmuls
    against the resident weight tiles.  PSUM accumulates over each block's
    contraction dim; results are copied to an SBUF staging tile and DMA'd
    out in one 2 MiB transfer per 128-row tile.
  - fp32 bits are fed directly into float32r tiles (verified bit-identical
    to explicitly rounded operands on HW - the PE rounds internally).
"""

import numpy as np

import concourse.bacc as bacc
import concourse.mybir as mybir
from concourse.tile import TileContext
from concourse.bass_utils import run_bass_kernel_spmd
from concourse.masks import make_identity

SIZES = [512, 1024, 256, 768, 384, 640, 128, 384]
OFFS = np.cumsum([0] + SIZES)
N_CORES = 8
ROWS_TOTAL = 4 * 2048
ROWS_PER_CORE = ROWS_TOTAL // N_CORES  # 1024
D = 4096
P = 128
R_TILES = ROWS_PER_CORE // P  # 8

# e-slices per block: chunks <=512, all >=256 when possible (f32r matmul
# runs 1 cycle/row only for moving dim >= 256; 512 is the PSUM bank limit)
E_SLICES = {
    512: [512], 1024: [512, 512], 256: [256], 768: [512, 256],
    384: [384], 640: [384, 256], 128: [128],
}

F32R = mybir.dt.float32r
F32 = mybir.dt.float32

_cache = {}


def build_nc():
    if "nc" in _cache:
        return _cache["nc"]
    nc = bacc.Bacc()
    x_d = nc.declare_dram_parameter("x", [ROWS_PER_CORE, D], F32R, isOutput=False)
    w_d = [
        nc.declare_dram_parameter(f"w{i}", [s, s], F32R, isOutput=False)
        for i, s in enumerate(SIZES)
    ]
    out_d = nc.declare_dram_parameter("out", [ROWS_PER_CORE, D], F32, isOutput=True)

    x_v = x_d.rearrange("(r p) d -> r p d", p=P)
    out_v = out_d.rearrange("(r p) d -> r p d", p=P)

    with TileContext(nc) as tc:
        with (
            tc.tile_pool(name="wres", bufs=1) as wres,
            tc.tile_pool(name="xnat", bufs=2) as xnat_p,
            tc.tile_pool(name="xt", bufs=3) as xt_p,
            tc.tile_pool(name="osb", bufs=2) as osb_p,
            tc.tile_pool(name="idp", bufs=1) as idp,
            tc.tile_pool(name="tp", bufs=2, space="PSUM") as tp_p,
            tc.tile_pool(name="mm", bufs=4, space="PSUM") as mm_p,
        ):
            # identity (f32r) for PE transpose
            id32 = idp.tile([P, P], F32, tag="id32")
            make_identity(nc, id32[:])
            ident = idp.tile([P, P], F32R, tag="idr")
            nc.vector.tensor_copy(ident[:], id32[:])

            # resident weights: per block, per k-tile: [128, s] f32r.
            # Issued on the Scalar-engine HWDGE ring so they stream in
            # parallel with x/out DMAs on the Sync ring (the SDMA engines
            # round-robin between the two rings).
            wt = []
            for i, s in enumerate(SIZES):
                w_v = w_d[i].rearrange("(k p) e -> k p e", p=P)
                ks = []
                for k in range(s // P):
                    t = wres.tile([P, s], F32R, tag=f"w{i}_{k}")
                    nc.scalar.dma_start(out=t[:], in_=w_v[k])
                    ks.append(t)
                wt.append(ks)

            # Software pipeline over row-tiles with lookahead L=2:
            # while row-tile r's matmuls run (first pass is gated on the
            # streaming weight DMAs), the transposes for row-tile r+2 are
            # interleaved between its blocks so the in-order TensorE stream
            # always has ready work during the weight preload.
            LOOKAHEAD = 2
            xnat = {}  # r -> (lo_tile, hi_tile)
            xts_all = {}  # r -> list of 8 xt tiles

            def issue_x_dma(r):
                lo = xnat_p.tile([P, D // 2], F32R, tag="xnl")
                hi = xnat_p.tile([P, D // 2], F32R, tag="xnh")
                nc.sync.dma_start(out=lo[:], in_=x_v[r][:, :D // 2])
                nc.sync.dma_start(out=hi[:], in_=x_v[r][:, D // 2:])
                xnat[r] = (lo, hi)

            def transpose_group(r, j):
                # transposes d-tiles 4j..4j+3 of row-tile r into xt tile j
                lo, hi = xnat[r]
                src = lo if j < 4 else hi
                base = P * 4 * j - (0 if j < 4 else D // 2)
                ps = tp_p.tile([P, 4 * P], F32R, tag="tp")
                for i in range(4):
                    nc.tensor.transpose(
                        ps[:, P * i:P * (i + 1)],
                        src[:, base + P * i:base + P * (i + 1)],
                        ident[:],
                    )
                xt = xt_p.tile([P, 4 * P], F32R, tag=f"xt{j}")
                nc.vector.tensor_copy(xt[:], ps[:])
                xts_all.setdefault(r, []).append(xt)

            for r in range(min(LOOKAHEAD, R_TILES)):
                issue_x_dma(r)
            for r in range(min(LOOKAHEAD, R_TILES)):
                for j in range(8):
                    transpose_group(r, j)

            for r in range(R_TILES):
                ra = r + LOOKAHEAD
                if ra < R_TILES:
                    issue_x_dma(ra)
                xts = xts_all[r]
                o_t = osb_p.tile([P, D], F32, tag="os")
                for b, s in enumerate(SIZES):
                    if ra < R_TILES:
                        transpose_group(ra, b)
                    d0 = int(OFFS[b]) // P  # first global d-tile of block
                    kt = s // P
                    # k-outer loop: consecutive matmuls share the same
                    # stationary operand (lhsT), one PSUM group per e-slice
                    slices = []
                    n0 = 0
                    for nw in E_SLICES[s]:
                        slices.append((n0, nw, mm_p.tile([P, nw], F32, tag="mm", name="mmps")))
                        n0 += nw
                    for k in range(kt):
                        g = d0 + k
                        lhsT = xts[g // 4][:, P * (g % 4):P * (g % 4 + 1)]
                        for n0, nw, ps in slices:
                            nc.tensor.matmul(
                                ps[:], lhsT, wt[b][k][:, n0:n0 + nw],
                                start=(k == 0), stop=(k == kt - 1),
                            )
                    # alternate PSUM->SBUF output copies between DVE and
                    # ACT so neither engine becomes the bottleneck
                    for n0, nw, ps in slices:
                        dst = o_t[:, int(OFFS[b]) + n0:int(OFFS[b]) + n0 + nw]
                        if (r + b) % 2 == 0:
                            nc.scalar.copy(dst, ps[:])
                        else:
                            nc.vector.tensor_copy(dst, ps[:])
                del xts_all[r]
                nc.sync.dma_start(out=out_v[r][:, :D // 2], in_=o_t[:, :D // 2])
                nc.sync.dma_start(out=out_v[r][:, D // 2:], in_=o_t[:, D // 2:])

    nc.finalize()
    _cache["nc"] = nc
    return nc


def build_in_maps(x, w0, w1, w2, w3, w4, w5, w6, w7):
    x = np.ascontiguousarray(np.asarray(x, dtype=np.float32)).reshape(ROWS_TOTAL, D)
    ws = [w0, w1, w2, w3, w4, w5, w6, w7]
    wts = [
        np.ascontiguousarray(np.asarray(w, dtype=np.float32).T) for w in ws
    ]
    in_maps = []
    for c in range(N_CORES):
        m = {"x": x[c * ROWS_PER_CORE:(c + 1) * ROWS_PER_CORE]}
        for i, wt in enumerate(wts):
            m[f"w{i}"] = wt
        in_maps.append(m)
    return in_maps


def kernel(x, w0, w1, w2, w3, w4, w5, w6, w7):
    nc = build_nc()
    in_maps = build_in_maps(x, w0, w1, w2, w3, w4, w5, w6, w7)
    res = run_bass_kernel_spmd(nc, in_maps, list(range(N_CORES)))
    out = np.concatenate([r["out"] for r in res.results], axis=0)
    return out.reshape(4, 2048, D).astype(np.float32, copy=False)


# revision 14
# speedup vs baseline: 1.0085x; 1.0085x over previous
"""Block-diagonal rotation (COB) kernel for Trainium2, 8 NeuronCores.

Computes out[..., block_i] = x[..., block_i] @ W_i.T for 8 square blocks of
sizes [512, 1024, 256, 768, 384, 640, 128, 384] (features sum to 4096),
x shape (4, 2048, 4096) fp32.

Strategy:
  - Pure data-parallel over rows: 8192 rows are split 8 ways (1024 rows/core).
    Each core gets all (host-pre-transposed) weights.
  - Weights are DMA'd once into SBUF and stay resident as float32r
    (TRN2's fast 4-byte matmul dtype: 1 cycle/row vs 4 for fp32,
    ~1.5e-4 max rel err at these contraction depths).
  - x tiles [128, 4096] are DMA'd naturally (rows on partitions), transposed
    128x128 on the TensorEngine (transpose mode), PSUM->SBUF copied by the
    VectorEngine, then used as the stationary operand of f32r matmuls
    against the resident weight tiles.  PSUM accumulates over each block's
    contraction dim; results are copied to an SBUF staging tile and DMA'd
    out in one 2 MiB transfer per 128-row tile.
  - fp32 bits are fed directly into float32r tiles (verified bit-identical
    to explicitly rounded operands on HW - the PE rounds internally).
"""

import numpy as np

import concourse.bacc as bacc
import concourse.mybir as mybir
from concourse.tile import TileContext
from concourse.bass_utils import run_bass_kernel_spmd
from concourse.masks import make_identity

SIZES = [512, 1024, 256, 768, 384, 640, 128, 384]
OFFS = np.cumsum([0] + SIZES)
N_CORES = 8
ROWS_TOTAL = 4 * 2048
ROWS_PER_CORE = ROWS_TOTAL // N_CORES  # 1024
D = 4096
P = 128
R_TILES = ROWS_PER_CORE // P  # 8

# e-slices per block: chunks <=512, all >=256 when possible (f32r matmul
# runs 1 cycle/row only for moving dim >= 256; 512 is the PSUM bank limit)
E_SLICES = {
    512: [512], 1024: [512, 512], 256: [256], 768: [512, 256],
    384: [384], 640: [384, 256], 128: [128],
}

F32R = mybir.dt.float32r
F32 = mybir.dt.float32

_cache = {}


def build_nc():
    if "nc" in _cache:
        return _cache["nc"]
    nc = bacc.Bacc()
    x_d = nc.declare_dram_parameter("x", [ROWS_PER_CORE, D], F32R, isOutput=False)
    w_d = [
        nc.declare_dram_parameter(f"w{i}", [s, s], F32R, isOutput=False)
        for i, s in enumerate(SIZES)
    ]
    out_d = nc.declare_dram_parameter("out", [ROWS_PER_CORE, D], F32, isOutput=True)

    x_v = x_d.rearrange("(r p) d -> r p d", p=P)
    out_v = out_d.rearrange("(r p) d -> r p d", p=P)

    with TileContext(nc) as tc:
        with (
            tc.tile_pool(name="wres", bufs=1) as wres,
            tc.tile_pool(name="xnat", bufs=2) as xnat_p,
            tc.tile_pool(name="xt", bufs=3) as xt_p,
            tc.tile_pool(name="osb", bufs=2) as osb_p,
            tc.tile_pool(name="idp", bufs=1) as idp,
            tc.tile_pool(name="tp", bufs=2, space="PSUM") as tp_p,
            tc.tile_pool(name="mm", bufs=4, space="PSUM") as mm_p,
        ):
            # identity (f32r) for PE transpose
            id32 = idp.tile([P, P], F32, tag="id32")
            make_identity(nc, id32[:])
            ident = idp.tile([P, P], F32R, tag="idr")
            nc.vector.tensor_copy(ident[:], id32[:])

            # resident weights: per block, per k-tile: [128, s] f32r.
            # Issued on the Scalar-engine HWDGE ring so they stream in
            # parallel with x/out DMAs on the Sync ring (the SDMA engines
            # round-robin between the two rings).
            wt = []
            for i, s in enumerate(SIZES):
                w_v = w_d[i].rearrange("(k p) e -> k p e", p=P)
                ks = []
                for k in range(s // P):
                    t = wres.tile([P, s], F32R, tag=f"w{i}_{k}")
                    nc.scalar.dma_start(out=t[:], in_=w_v[k])
                    ks.append(t)
                wt.append(ks)

            # Software pipeline over row-tiles with lookahead L=2:
            # while row-tile r's matmuls run (first pass is gated on the
            # streaming weight DMAs), the transposes for row-tile r+2 are
            # interleaved between its blocks so the in-order TensorE stream
            # always has ready work during the weight preload.
            LOOKAHEAD = 2
            xnat = {}  # r -> (lo_tile, hi_tile)
            xts_all = {}  # r -> list of 8 xt tiles

            def issue_x_dma(r):
                lo = xnat_p.tile([P, D // 2], F32R, tag="xnl")
                hi = xnat_p.tile([P, D // 2], F32R, tag="xnh")
                nc.sync.dma_start(out=lo[:], in_=x_v[r][:, :D // 2])
                nc.sync.dma_start(out=hi[:], in_=x_v[r][:, D // 2:])
                xnat[r] = (lo, hi)

            def transpose_group(r, j):
                # transposes d-tiles 4j..4j+3 of row-tile r into xt tile j
                lo, hi = xnat[r]
                src = lo if j < 4 else hi
                base = P * 4 * j - (0 if j < 4 else D // 2)
                ps = tp_p.tile([P, 4 * P], F32R, tag="tp")
                for i in range(4):
                    nc.tensor.transpose(
                        ps[:, P * i:P * (i + 1)],
                        src[:, base + P * i:base + P * (i + 1)],
                        ident[:],
                    )
                xt = xt_p.tile([P, 4 * P], F32R, tag=f"xt{j}")
                nc.vector.tensor_copy(xt[:], ps[:])
                xts_all.setdefault(r, []).append(xt)

            for r in range(min(LOOKAHEAD, R_TILES)):
                issue_x_dma(r)
            for r in range(min(LOOKAHEAD, R_TILES)):
                for j in range(8):
                    transpose_group(r, j)

            for r in range(R_TILES):
                ra = r + LOOKAHEAD
                if ra < R_TILES:
                    issue_x_dma(ra)
                xts = xts_all[r]
                o_t = osb_p.tile([P, D], F32, tag="os")
                for b, s in enumerate(SIZES):
                    if ra < R_TILES:
                        transpose_group(ra, b)
                    d0 = int(OFFS[b]) // P  # first global d-tile of block
                    kt = s // P
                    n0 = 0
                    for nw in E_SLICES[s]:
                        ps = mm_p.tile([P, nw], F32, tag="mm", name="mmps")
                        for k in range(kt):
                            g = d0 + k
                            lhsT = xts[g // 4][:, P * (g % 4):P * (g % 4 + 1)]
                            nc.tensor.matmul(
                                ps[:], lhsT, wt[b][k][:, n0:n0 + nw],
                                start=(k == 0), stop=(k == kt - 1),
                            )
                        # alternate PSUM->SBUF output copies between DVE and
                        # ACT so neither engine becomes the bottleneck
                        dst = o_t[:, int(OFFS[b]) + n0:int(OFFS[b]) + n0 + nw]
                        if (r + b) % 2 == 0:
                            nc.scalar.copy(dst, ps[:])
                        else:
                            nc.vector.tensor_copy(dst, ps[:])
                        n0 += nw
                del xts_all[r]
                nc.sync.dma_start(out=out_v[r][:, :D // 2], in_=o_t[:, :D // 2])
                nc.sync.dma_start(out=out_v[r][:, D // 2:], in_=o_t[:, D // 2:])

    nc.finalize()
    _cache["nc"] = nc
    return nc


def build_in_maps(x, w0, w1, w2, w3, w4, w5, w6, w7):
    x = np.ascontiguousarray(np.asarray(x, dtype=np.float32)).reshape(ROWS_TOTAL, D)
    ws = [w0, w1, w2, w3, w4, w5, w6, w7]
    wts = [
        np.ascontiguousarray(np.asarray(w, dtype=np.float32).T) for w in ws
    ]
    in_maps = []
    for c in range(N_CORES):
        m = {"x": x[c * ROWS_PER_CORE:(c + 1) * ROWS_PER_CORE]}
        for i, wt in enumerate(wts):
            m[f"w{i}"] = wt
        in_maps.append(m)
    return in_maps


def kernel(x, w0, w1, w2, w3, w4, w5, w6, w7):
    nc = build_nc()
    in_maps = build_in_maps(x, w0, w1, w2, w3, w4, w5, w6, w7)
    res = run_bass_kernel_spmd(nc, in_maps, list(range(N_CORES)))
    out = np.concatenate([r["out"] for r in res.results], axis=0)
    return out.reshape(4, 2048, D).astype(np.float32, copy=False)


# revision 16
# speedup vs baseline: 1.1877x; 1.1777x over previous
"""Block-diagonal rotation (COB) kernel for Trainium2, 8 NeuronCores.

Computes out[..., block_i] = x[..., block_i] @ W_i.T for 8 square blocks of
sizes [512, 1024, 256, 768, 384, 640, 128, 384] (features sum to 4096),
x shape (4, 2048, 4096) fp32.

Strategy:
  - Pure data-parallel over rows: 8192 rows are split 8 ways (1024 rows/core).
    Each core gets all (host-pre-transposed) weights.
  - Weights are DMA'd once into SBUF and stay resident as float32r
    (TRN2's fast 4-byte matmul dtype: 1 cycle/row vs 4 for fp32,
    ~1.5e-4 max rel err at these contraction depths).
  - x tiles [128, 4096] are DMA'd naturally (rows on partitions), transposed
    128x128 on the TensorEngine (transpose mode), PSUM->SBUF copied by the
    VectorEngine, then used as the stationary operand of f32r matmuls
    against the resident weight tiles.  PSUM accumulates over each block's
    contraction dim; results are copied to an SBUF staging tile and DMA'd
    out in one 2 MiB transfer per 128-row tile.
  - fp32 bits are fed directly into float32r tiles (verified bit-identical
    to explicitly rounded operands on HW - the PE rounds internally).
"""

import numpy as np

import concourse.bacc as bacc
import concourse.mybir as mybir
from concourse.tile import TileContext
from concourse.bass_utils import run_bass_kernel_spmd
from concourse.masks import make_identity

SIZES = [512, 1024, 256, 768, 384, 640, 128, 384]
OFFS = np.cumsum([0] + SIZES)
N_CORES = 8
ROWS_TOTAL = 4 * 2048
ROWS_PER_CORE = ROWS_TOTAL // N_CORES  # 1024
D = 4096
P = 128
R_TILES = ROWS_PER_CORE // P  # 8

# e-slices per block: chunks <=512, all >=256 when possible (f32r matmul
# runs 1 cycle/row only for moving dim >= 256; 512 is the PSUM bank limit)
E_SLICES = {
    512: [512], 1024: [512, 512], 256: [256], 768: [512, 256],
    384: [384], 640: [384, 256], 128: [128],
}

F32R = mybir.dt.float32r
F32 = mybir.dt.float32

_cache = {}


def build_nc():
    if "nc" in _cache:
        return _cache["nc"]
    nc = bacc.Bacc()
    x_d = nc.declare_dram_parameter("x", [ROWS_PER_CORE, D], F32R, isOutput=False)
    w_d = [
        nc.declare_dram_parameter(f"w{i}", [s, s], F32R, isOutput=False)
        for i, s in enumerate(SIZES)
    ]
    out_d = nc.declare_dram_parameter("out", [ROWS_PER_CORE, D], F32, isOutput=True)

    x_v = x_d.rearrange("(r p) d -> r p d", p=P)
    out_v = out_d.rearrange("(r p) d -> r p d", p=P)

    with TileContext(nc) as tc:
        with (
            tc.tile_pool(name="wres", bufs=1) as wres,
            tc.tile_pool(name="xnat", bufs=2) as xnat_p,
            tc.tile_pool(name="xt", bufs=3) as xt_p,
            tc.tile_pool(name="osb", bufs=2) as osb_p,
            tc.tile_pool(name="idp", bufs=1) as idp,
            tc.tile_pool(name="tp", bufs=2, space="PSUM") as tp_p,
            tc.tile_pool(name="mm", bufs=4, space="PSUM") as mm_p,
        ):
            # identity (f32r) for PE transpose
            id32 = idp.tile([P, P], F32, tag="id32")
            make_identity(nc, id32[:])
            ident = idp.tile([P, P], F32R, tag="idr")
            nc.vector.tensor_copy(ident[:], id32[:])

            # resident weights: per block, per k-tile: [128, s] f32r.
            # Even-numbered chunks stream on the Scalar-engine HWDGE ring
            # immediately; odd-numbered chunks go on the Sync ring, queued
            # right after the prologue x tiles (deferred emission below), so
            # the weight preload finishes roughly twice as fast while the
            # early x tiles still arrive first.
            wt = []
            w_sync_dmas = []  # (tile, src) deferred to after prologue x DMAs
            ci = 0
            for i, s in enumerate(SIZES):
                w_v = w_d[i].rearrange("(k p) e -> k p e", p=P)
                ks = []
                for k in range(s // P):
                    t = wres.tile([P, s], F32R, tag=f"w{i}_{k}")
                    if ci % 2 == 0:
                        nc.scalar.dma_start(out=t[:], in_=w_v[k])
                    else:
                        w_sync_dmas.append((t, w_v[k]))
                    ks.append(t)
                    ci += 1
                wt.append(ks)

            # Software pipeline over row-tiles with lookahead L=2:
            # while row-tile r's matmuls run (first pass is gated on the
            # streaming weight DMAs), the transposes for row-tile r+2 are
            # interleaved between its blocks so the in-order TensorE stream
            # always has ready work during the weight preload.
            LOOKAHEAD = 2
            xnat = {}  # r -> (lo_tile, hi_tile)
            xts_all = {}  # r -> list of 8 xt tiles

            def issue_x_dma(r):
                lo = xnat_p.tile([P, D // 2], F32R, tag="xnl")
                hi = xnat_p.tile([P, D // 2], F32R, tag="xnh")
                nc.sync.dma_start(out=lo[:], in_=x_v[r][:, :D // 2])
                nc.sync.dma_start(out=hi[:], in_=x_v[r][:, D // 2:])
                xnat[r] = (lo, hi)

            def transpose_group(r, j):
                # transposes d-tiles 4j..4j+3 of row-tile r into xt tile j
                lo, hi = xnat[r]
                src = lo if j < 4 else hi
                base = P * 4 * j - (0 if j < 4 else D // 2)
                ps = tp_p.tile([P, 4 * P], F32R, tag="tp")
                for i in range(4):
                    nc.tensor.transpose(
                        ps[:, P * i:P * (i + 1)],
                        src[:, base + P * i:base + P * (i + 1)],
                        ident[:],
                    )
                xt = xt_p.tile([P, 4 * P], F32R, tag=f"xt{j}")
                nc.vector.tensor_copy(xt[:], ps[:])
                xts_all.setdefault(r, []).append(xt)

            for r in range(min(LOOKAHEAD, R_TILES)):
                issue_x_dma(r)
            for t, src in w_sync_dmas:
                nc.sync.dma_start(out=t[:], in_=src)
            for r in range(min(LOOKAHEAD, R_TILES)):
                for j in range(8):
                    transpose_group(r, j)

            for r in range(R_TILES):
                ra = r + LOOKAHEAD
                if ra < R_TILES:
                    issue_x_dma(ra)
                xts = xts_all[r]
                o_t = osb_p.tile([P, D], F32, tag="os")
                for b, s in enumerate(SIZES):
                    if ra < R_TILES:
                        transpose_group(ra, b)
                    d0 = int(OFFS[b]) // P  # first global d-tile of block
                    kt = s // P
                    n0 = 0
                    for nw in E_SLICES[s]:
                        ps = mm_p.tile([P, nw], F32, tag="mm", name="mmps")
                        for k in range(kt):
                            g = d0 + k
                            lhsT = xts[g // 4][:, P * (g % 4):P * (g % 4 + 1)]
                            nc.tensor.matmul(
                                ps[:], lhsT, wt[b][k][:, n0:n0 + nw],
                                start=(k == 0), stop=(k == kt - 1),
                            )
                        # alternate PSUM->SBUF output copies between DVE and
                        # ACT so neither engine becomes the bottleneck
                        dst = o_t[:, int(OFFS[b]) + n0:int(OFFS[b]) + n0 + nw]
                        if (r + b) % 2 == 0:
                            nc.scalar.copy(dst, ps[:])
                        else:
                            nc.vector.tensor_copy(dst, ps[:])
                        n0 += nw
                del xts_all[r]
                nc.sync.dma_start(out=out_v[r][:, :D // 2], in_=o_t[:, :D // 2])
                nc.sync.dma_start(out=out_v[r][:, D // 2:], in_=o_t[:, D // 2:])

    nc.finalize()
    _cache["nc"] = nc
    return nc


def build_in_maps(x, w0, w1, w2, w3, w4, w5, w6, w7):
    x = np.ascontiguousarray(np.asarray(x, dtype=np.float32)).reshape(ROWS_TOTAL, D)
    ws = [w0, w1, w2, w3, w4, w5, w6, w7]
    wts = [
        np.ascontiguousarray(np.asarray(w, dtype=np.float32).T) for w in ws
    ]
    in_maps = []
    for c in range(N_CORES):
        m = {"x": x[c * ROWS_PER_CORE:(c + 1) * ROWS_PER_CORE]}
        for i, wt in enumerate(wts):
            m[f"w{i}"] = wt
        in_maps.append(m)
    return in_maps


def kernel(x, w0, w1, w2, w3, w4, w5, w6, w7):
    nc = build_nc()
    in_maps = build_in_maps(x, w0, w1, w2, w3, w4, w5, w6, w7)
    res = run_bass_kernel_spmd(nc, in_maps, list(range(N_CORES)))
    out = np.concatenate([r["out"] for r in res.results], axis=0)
    return out.reshape(4, 2048, D).astype(np.float32, copy=False)
